# revision 9
# baseline (speedup 1.0000x reference)
"""CPC loss kernel for Trainium2, batch-sharded across 8 NeuronCores.

Shapes (hardcoded per problem spec):
  z, c: [2048, 64, 128] f32;  mask, neg_map: [128, 64] int;  W: [128, 128] f32
  ln_weight/ln_bias: [128] f32.  Output: scalar f32.

Per-core plan (Bc = 8 batch elements):
  - Host packs per-core flat row tables zf/cf [SEQ*Bc, 128] plus an int32
    meta tensor holding gather indices and the neg-row keep multiplier.
  - Device gathers only the 3*L*Bc needed rows via indirect DMA (the whole
    point: ~1.5MB read instead of 16MB streamed per core).
  - LN stats along the free dim, (z-mu)*rstd with keep folded into rstd.
  - PE transposes + two fp32r matmul stages give MT[j, i] = M[i, j, b].
  - exp on ACT with accum_out produces the softmax denominator; the diagonal
    logit comes from an elementwise product with a ones-matmul reduction.
  - Per-core output is sum_{j,b} log(p_diag + 1e-3); host combines.

ln_weight is folded into W on the host; ln_bias adds a per-(j,b) constant to
every logit column and cancels in the softmax, so it is dropped.
"""

import numpy as np

SEQ, B, L, ZD, CD = 2048, 64, 128, 128, 128
NCORES = 8
BC = B // NCORES  # 8
NSEG = 2 * BC  # 16 LN segments per core (pos + neg)
LN_EPS = 1e-5

_cached = None


def _build_program():
    import concourse.bacc as bacc
    import concourse.tile as tile
    from concourse import bass, mybir
    from concourse.masks import make_identity

    f32 = mybir.dt.float32
    f32r = mybir.dt.float32r
    i32 = mybir.dt.int32
    AF = mybir.ActivationFunctionType
    ALU = mybir.AluOpType
    AX = mybir.AxisListType

    nc = bacc.Bacc(
        "TRN2",
        target_bir_lowering=False,
        debug=False,
        enable_asserts=True,
        num_devices=NCORES,
    )

    zf_d = nc.dram_tensor("zf", [SEQ * BC, ZD], f32, kind="ExternalInput")
    cf_d = nc.dram_tensor("cf", [SEQ * BC, CD], f32, kind="ExternalInput")
    # meta: [:,0:8]=pos idx, [:,8:16]=neg idx, [:,16:24]=keep (f32 bits)
    meta_d = nc.dram_tensor("meta", [L, 3 * BC], i32, kind="ExternalInput")
    wt_d = nc.dram_tensor("wt", [CD, ZD], f32r, kind="ExternalInput")
    out_d = nc.dram_tensor("out", [1, 1], f32, kind="ExternalOutput")

    with tile.TileContext(nc) as tc:
        with (
            tc.tile_pool(name="singles", bufs=1) as singles,
            tc.tile_pool(name="scratch", bufs=2) as scratch,
            tc.tile_pool(name="pwide", bufs=2, space="PSUM") as pwide,
            tc.tile_pool(name="pmt", bufs=3, space="PSUM") as pmt,
            tc.tile_pool(name="pcol", bufs=2, space="PSUM") as pcol,
        ):
            # ---- constants and small inputs ----
            meta_sb = singles.tile([L, 3 * BC], i32)
            nc.sync.dma_start(meta_sb[:], meta_d.ap())
            wt_sb = singles.tile([CD, ZD], f32r)
            nc.sync.dma_start(wt_sb[:], wt_d.ap())
            ident = singles.tile([128, 128], f32)
            make_identity(nc, ident[:])
            ones = singles.tile([128, 1], f32)
            nc.vector.memset(ones[:], 1.0)
            eps_sb = singles.tile([128, 1], f32)
            nc.vector.memset(eps_sb[:], LN_EPS)
            c3_sb = singles.tile([128, 1], f32)
            nc.vector.memset(c3_sb[:], 1e-3)

            pos_idx = meta_sb[:, 0:BC]
            neg_idx = meta_sb[:, BC : 2 * BC]
            keep = meta_sb[:, 2 * BC : 3 * BC].bitcast(f32)

            # ---- gathers ----
            zall = singles.tile([128, NSEG * ZD], f32)  # [j | b*128+z], pos then neg
            call = singles.tile([128, BC * CD], f32)
            nc.gpsimd.indirect_dma_start(
                out=call[:],
                out_offset=None,
                in_=cf_d.ap(),
                in_offset=bass.IndirectOffsetOnAxis(ap=pos_idx, axis=0),
            )
            nc.gpsimd.indirect_dma_start(
                out=zall[:, 0 : BC * ZD],
                out_offset=None,
                in_=zf_d.ap(),
                in_offset=bass.IndirectOffsetOnAxis(ap=pos_idx, axis=0),
            )
            nc.gpsimd.indirect_dma_start(
                out=zall[:, BC * ZD : NSEG * ZD],
                out_offset=None,
                in_=zf_d.ap(),
                in_offset=bass.IndirectOffsetOnAxis(ap=neg_idx, axis=0),
            )

            # ---- c-side: transpose ct per batch, batched E = W' @ ct^T ----
            # ctT[c, b*128+j]; E[z, b*128+j] = sum_c wt[c,z] * ctT[c, j(b)]
            ctT_sb = singles.tile([CD, BC * L], f32r)
            for g in range(2):
                pct = pwide.tile([128, 4 * 128], f32, tag="pw")
                for k in range(4):
                    b = g * 4 + k
                    nc.tensor.transpose(
                        out=pct[:, k * 128 : (k + 1) * 128],
                        in_=call[:, b * CD : (b + 1) * CD],
                        identity=ident[:],
                    )
                nc.scalar.copy(ctT_sb[:, g * 512 : (g + 1) * 512], pct[:])
            e_sb = singles.tile([ZD, BC * L], f32r)
            for g in range(2):
                pe = pwide.tile([128, 512], f32, tag="pw")
                nc.tensor.matmul(
                    out=pe[:],
                    lhsT=wt_sb[:],
                    rhs=ctT_sb[:, g * 512 : (g + 1) * 512],
                    start=True,
                    stop=True,
                )
                nc.vector.tensor_copy(e_sb[:, g * 512 : (g + 1) * 512], pe[:])

            # ---- layernorm over z segments ----
            z3 = zall[:].rearrange("p (s d) -> p s d", d=ZD)
            s1 = singles.tile([128, NSEG], f32)
            nc.vector.reduce_sum(out=s1[:], in_=z3, axis=AX.X)
            sq = singles.tile([128, NSEG * ZD], f32)
            nc.scalar.activation(sq[:], zall[:], AF.Square)
            s2 = singles.tile([128, NSEG], f32)
            nc.vector.reduce_sum(
                out=s2[:], in_=sq[:].rearrange("p (s d) -> p s d", d=ZD), axis=AX.X
            )
            mu = singles.tile([128, NSEG], f32)
            nc.scalar.mul(mu[:], s1[:], 1.0 / ZD)
            musq = singles.tile([128, NSEG], f32)
            nc.vector.tensor_mul(musq[:], mu[:], mu[:])
            var = singles.tile([128, NSEG], f32)
            nc.scalar.mul(var[:], s2[:], 1.0 / ZD)
            nc.vector.tensor_tensor(
                out=var[:], in0=var[:], in1=musq[:], op=ALU.subtract
            )
            std = singles.tile([128, NSEG], f32)
            nc.scalar.activation(std[:], var[:], AF.Sqrt, bias=eps_sb[:])
            rstd = singles.tile([128, NSEG], f32)
            nc.vector.reciprocal(rstd[:], std[:])
            # zero out masked negative rows by folding keep into rstd
            nc.vector.tensor_mul(rstd[:, BC:NSEG], rstd[:, BC:NSEG], keep)

            zln = singles.tile([128, NSEG * ZD], f32)
            zl3 = zln[:].rearrange("p (s d) -> p s d", d=ZD)
            mu_bc = mu[:].unsqueeze(-1).to_broadcast([128, NSEG, ZD])
            rstd_bc = rstd[:].unsqueeze(-1).to_broadcast([128, NSEG, ZD])
            nc.vector.tensor_tensor(out=zl3, in0=z3, in1=mu_bc, op=ALU.subtract)
            nc.vector.tensor_tensor(out=zl3, in0=zl3, in1=rstd_bc, op=ALU.mult)

            # ---- per-batch: MT = (E_b)^T-contraction over z; softmax pieces ----
            den = singles.tile([128, BC], f32)
            diagm = singles.tile([128, BC], f32)
            for b in range(BC):
                pzt = pmt.tile([128, 256], f32, tag="pm")
                nc.tensor.transpose(
                    out=pzt[:, 0:128],
                    in_=zln[:, b * ZD : (b + 1) * ZD],
                    identity=ident[:],
                )
                nc.tensor.transpose(
                    out=pzt[:, 128:256],
                    in_=zln[:, (BC + b) * ZD : (BC + b + 1) * ZD],
                    identity=ident[:],
                )
                zt_sb = scratch.tile([128, 256], f32r, tag="zt")
                nc.scalar.copy(zt_sb[:], pzt[:])

                pmt_b = pmt.tile([128, 256], f32, tag="pm")
                nc.tensor.matmul(
                    out=pmt_b[:],
                    lhsT=e_sb[:, b * L : (b + 1) * L],
                    rhs=zt_sb[:],
                    start=True,
                    stop=True,
                )
                expt = scratch.tile([128, 256], f32, tag="expt")
                nc.scalar.activation(
                    expt[:], pmt_b[:], AF.Exp, accum_out=den[:, b : b + 1]
                )

                # diag: M[j,j,b] = sum_z E[z,j] * zT_pos[z,j]
                dz = scratch.tile([128, 128], f32, tag="dz")
                nc.vector.tensor_mul(
                    dz[:],
                    e_sb[:, b * L : (b + 1) * L].bitcast(f32),
                    zt_sb[:, 0:128].bitcast(f32),
                )
                pdg = pcol.tile([128, 1], f32, tag="pc")
                nc.tensor.matmul(
                    out=pdg[:], lhsT=dz[:], rhs=ones[:], start=True, stop=True
                )
                nc.scalar.copy(diagm[:, b : b + 1], pdg[:])

            # ---- loss tail: p = exp(diag)/den; sum log(p + 1e-3) ----
            num = singles.tile([128, BC], f32)
            nc.scalar.activation(num[:], diagm[:], AF.Exp)
            rden = singles.tile([128, BC], f32)
            nc.vector.reciprocal(rden[:], den[:])
            prob = singles.tile([128, BC], f32)
            nc.vector.tensor_mul(prob[:], num[:], rden[:])
            logp = singles.tile([128, BC], f32)
            nc.scalar.activation(logp[:], prob[:], AF.Ln, bias=c3_sb[:])
            colred = singles.tile([128, 1], f32)
            nc.vector.reduce_sum(out=colred[:], in_=logp[:], axis=AX.X)
            ptot = pcol.tile([1, 1], f32, tag="pc")
            nc.tensor.matmul(
                out=ptot[:], lhsT=colred[:], rhs=ones[:], start=True, stop=True
            )
            out_sb = singles.tile([1, 1], f32)
            nc.vector.tensor_copy(out_sb[:], ptot[:])
            nc.sync.dma_start(out_d.ap(), out_sb[:])

    nc.compile()
    return nc


def _prep_in_maps(z, c, mask, neg_map, W, ln_weight):
    z = np.asarray(z, dtype=np.float32)
    c = np.asarray(c, dtype=np.float32)
    mask = np.asarray(mask).astype(np.int64)
    neg_map = np.asarray(neg_map).astype(np.int64)
    W = np.asarray(W, dtype=np.float32)
    ln_weight = np.asarray(ln_weight, dtype=np.float32)

    wt = np.ascontiguousarray((ln_weight[:, None] * W).T)  # [c, z] = W'[z, c]
    boff = np.arange(BC, dtype=np.int64)[None, :]
    in_maps = []
    for i in range(NCORES):
        bsl = slice(i * BC, (i + 1) * BC)
        zf = np.ascontiguousarray(z[:, bsl, :]).reshape(SEQ * BC, ZD)
        cf = np.ascontiguousarray(c[:, bsl, :]).reshape(SEQ * BC, CD)
        m = mask[:, bsl]
        n = neg_map[:, bsl]
        pos_idx = (m * BC + boff).astype(np.int32)
        neg_idx = (n * BC + boff).astype(np.int32)
        hit = (n[:, None, :] == m[None, :, :]).any(axis=1)  # [L, BC]
        keep = (~hit).astype(np.float32)
        meta = np.concatenate(
            [pos_idx, neg_idx, keep.view(np.int32)], axis=1
        ).astype(np.int32)
        in_maps.append({"zf": zf, "cf": cf, "meta": meta, "wt": wt})
    return in_maps


def kernel(z, c, mask, neg_map, W, ln_weight, ln_bias):
    from concourse import bass_utils

    global _cached
    if _cached is None:
        _cached = _build_program()
    nc = _cached

    in_maps = _prep_in_maps(z, c, mask, neg_map, W, ln_weight)
    res = bass_utils.run_bass_kernel_spmd(
        nc, in_maps, core_ids=list(range(NCORES))
    )
    total = np.float64(0.0)
    for i in range(NCORES):
        total += np.float64(res.results[i]["out"][0, 0])
    return np.float32(-(total / (L * B)))


# revision 15
# speedup vs baseline: 1.0092x; 1.0092x over previous
"""CPC loss kernel for Trainium2, batch-sharded across 8 NeuronCores.

Shapes (hardcoded per problem spec):
  z, c: [2048, 64, 128] f32;  mask, neg_map: [128, 64] int;  W: [128, 128] f32
  ln_weight/ln_bias: [128] f32.  Output: scalar f32.

Per-core plan (Bc = 8 batch elements):
  - Host packs per-core flat row tables zf/cf [SEQ*Bc, 128] plus an int32
    meta tensor holding gather indices and the neg-row keep multiplier.
  - Device gathers only the 3*L*Bc needed rows via indirect DMA (~1.5MB read
    instead of 16MB streamed per core).
  - LN stats per 128-wide segment via bn_stats/bn_aggr; normalization fused
    into one tensor_scalar per segment, split across DVE and GpSimd.
  - PE transposes + two fp32r matmul stages give MT[j, i] = M[i, j, b].
  - exp on ACT with accum_out produces the softmax denominator; the diagonal
    numerator is exp(MT)[:, :128] * I reduced along the free dim.
  - Per-core output is sum_{j,b} log(p_diag + 1e-3); host combines.

ln_weight is folded into W on the host; ln_bias adds a per-(j,b) constant to
every logit column and cancels in the softmax, so it is dropped.  The keep
multiplier (zeroing negatives that collide with mask positions) folds into
rstd.  Softmax needs no max-subtraction: |logits| < ~70 and exp stays well
inside f32 range.
"""

import numpy as np

SEQ, B, L, ZD, CD = 2048, 64, 128, 128, 128
NCORES = 8
BC = B // NCORES  # 8
NSEG = 2 * BC  # 16 LN segments per core (pos + neg)
LN_EPS = 1e-5
TR_F32R = True  # fp32r PE transposes (1.5 cyc/row vs 2.0 for fp32)

_cached = None


def _build_program():
    import concourse.bacc as bacc
    import concourse.tile as tile
    from concourse import bass, mybir
    from concourse.masks import make_identity

    f32 = mybir.dt.float32
    f32r = mybir.dt.float32r
    i32 = mybir.dt.int32
    AF = mybir.ActivationFunctionType
    ALU = mybir.AluOpType
    AX = mybir.AxisListType
    trdt = f32r if TR_F32R else f32

    nc = bacc.Bacc(
        "TRN2",
        target_bir_lowering=False,
        debug=False,
        enable_asserts=True,
        num_devices=NCORES,
    )

    zf_d = nc.dram_tensor("zf", [SEQ * BC, ZD], f32, kind="ExternalInput")
    cf_d = nc.dram_tensor("cf", [SEQ * BC, CD], f32, kind="ExternalInput")
    # meta: [:,0:8]=pos idx, [:,8:16]=neg idx, [:,16:24]=keep (f32 bits)
    meta_d = nc.dram_tensor("meta", [L, 3 * BC], i32, kind="ExternalInput")
    wt_d = nc.dram_tensor("wt", [CD, ZD], f32r, kind="ExternalInput")
    out_d = nc.dram_tensor("out", [1, 1], f32, kind="ExternalOutput")

    with tile.TileContext(nc) as tc:
        with (
            tc.tile_pool(name="singles", bufs=1) as singles,
            tc.tile_pool(name="scratch", bufs=2) as scratch,
            tc.tile_pool(name="pwide", bufs=2, space="PSUM") as pwide,
            tc.tile_pool(name="pzt", bufs=2, space="PSUM") as ppzt,
            tc.tile_pool(name="pmt", bufs=2, space="PSUM") as ppmt,
            tc.tile_pool(name="pcol", bufs=1, space="PSUM") as pcol,
        ):
            # ---- small inputs first: gathers depend on meta ----
            meta_sb = singles.tile([L, 3 * BC], i32)
            nc.sync.dma_start(meta_sb[:], meta_d.ap())
            wt_sb = singles.tile([CD, ZD], f32r)
            nc.sync.dma_start(wt_sb[:], wt_d.ap())

            pos_idx = meta_sb[:, 0:BC]
            neg_idx = meta_sb[:, BC : 2 * BC]
            keep = meta_sb[:, 2 * BC : 3 * BC].bitcast(f32)

            # ---- gathers (c first: it unblocks the PE ct-transpose path) ----
            zall = singles.tile([128, NSEG * ZD], f32)  # [j | seg*128+z]
            call = singles.tile([128, BC * CD], trdt)
            nc.gpsimd.indirect_dma_start(
                out=call[:],
                out_offset=None,
                in_=cf_d.ap(),
                in_offset=bass.IndirectOffsetOnAxis(ap=pos_idx, axis=0),
            )
            nc.gpsimd.indirect_dma_start(
                out=zall[:, 0 : BC * ZD],
                out_offset=None,
                in_=zf_d.ap(),
                in_offset=bass.IndirectOffsetOnAxis(ap=pos_idx, axis=0),
            )
            nc.gpsimd.indirect_dma_start(
                out=zall[:, BC * ZD : NSEG * ZD],
                out_offset=None,
                in_=zf_d.ap(),
                in_offset=bass.IndirectOffsetOnAxis(ap=neg_idx, axis=0),
            )

            # ---- constants ----
            ident = singles.tile([128, 128], f32)
            make_identity(nc, ident[:])
            ident_f32 = ident[:]
            if TR_F32R:
                ident_r = singles.tile([128, 128], f32r)
                nc.gpsimd.tensor_copy(ident_r[:], ident[:])
                ident_tr = ident_r[:]
            else:
                ident_tr = ident[:]
            ones = singles.tile([128, 1], f32)
            nc.vector.memset(ones[:], 1.0)
            eps_sb = singles.tile([128, 1], f32)
            nc.vector.memset(eps_sb[:], LN_EPS)
            c3_sb = singles.tile([128, 1], f32)
            nc.vector.memset(c3_sb[:], 1e-3)

            # ---- c-side: transpose ct per batch, batched E = W' @ ct^T ----
            ctT_sb = singles.tile([CD, BC * L], f32r)
            for g in range(2):
                pct = pwide.tile([128, 512], trdt, tag="pw")
                for k in range(4):
                    b = g * 4 + k
                    nc.tensor.transpose(
                        out=pct[:, k * 128 : (k + 1) * 128],
                        in_=call[:, b * CD : (b + 1) * CD],
                        identity=ident_tr,
                    )
                nc.vector.tensor_copy(
                    ctT_sb[:, g * 512 : (g + 1) * 512], pct[:].bitcast(f32r)
                )
            e_sb = singles.tile([ZD, BC * L], f32r)
            for g in range(2):
                pe = pwide.tile([128, 512], f32, tag="pw")
                nc.tensor.matmul(
                    out=pe[:],
                    lhsT=wt_sb[:],
                    rhs=ctT_sb[:, g * 512 : (g + 1) * 512],
                    start=True,
                    stop=True,
                )
                nc.vector.tensor_copy(e_sb[:, g * 512 : (g + 1) * 512], pe[:])

            # ---- layernorm stats per segment (bn_stats one-pass) ----
            z3 = zall[:].rearrange("p (s d) -> p s d", d=ZD)
            stats = singles.tile([128, NSEG, 6], f32)
            mv = singles.tile([128, NSEG, 2], f32)
            std = singles.tile([128, NSEG], f32)
            rstd = singles.tile([128, NSEG], f32)
            for half in range(2):
                segs = range(half * BC, (half + 1) * BC)
                for s in segs:
                    nc.vector.bn_stats(out=stats[:, s, :], in_=z3[:, s, :])
                for s in segs:
                    nc.vector.bn_aggr(out=mv[:, s, :], in_=stats[:, s, :])
                hsl = slice(half * BC, (half + 1) * BC)
                nc.scalar.activation(
                    std[:, hsl], mv[:, hsl, 1], AF.Sqrt, bias=eps_sb[:]
                )
                nc.vector.reciprocal(rstd[:, hsl], std[:, hsl])
            # zero out masked negative rows by folding keep into rstd
            nc.vector.tensor_mul(rstd[:, BC:NSEG], rstd[:, BC:NSEG], keep)

            # ---- normalize per segment: (z - mu) * rstd, DVE/GpSimd split ----
            zln = singles.tile([128, NSEG * ZD], trdt)
            zl3 = zln[:].rearrange("p (s d) -> p s d", d=ZD)
            for b in range(BC):
                for i, s in enumerate((b, BC + b)):
                    eng = nc.vector if i == 0 else nc.gpsimd
                    eng.tensor_scalar(
                        out=zl3[:, s, :],
                        in0=z3[:, s, :],
                        scalar1=mv[:, s, 0:1],
                        scalar2=rstd[:, s : s + 1],
                        op0=ALU.subtract,
                        op1=ALU.mult,
                    )

            # ---- per-batch: transposes, MT matmul, exp/den/num ----
            den = singles.tile([128, BC], f32)
            num = singles.tile([128, BC], f32)
            for b in range(BC):
                pzt = ppzt.tile([128, 256], trdt, tag="pzt")
                for i, s in enumerate((b, BC + b)):
                    nc.tensor.transpose(
                        out=pzt[:, i * 128 : (i + 1) * 128],
                        in_=zln[:, s * ZD : (s + 1) * ZD],
                        identity=ident_tr,
                    )
                zt_sb = scratch.tile([128, 256], f32r, tag="zt")
                nc.vector.tensor_copy(zt_sb[:], pzt[:].bitcast(f32r))

                pmt_b = ppmt.tile([128, 256], f32, tag="pmt")
                nc.tensor.matmul(
                    out=pmt_b[:],
                    lhsT=e_sb[:, b * L : (b + 1) * L],
                    rhs=zt_sb[:],
                    start=True,
                    stop=True,
                )
                expt = scratch.tile([128, 256], f32, tag="expt")
                nc.scalar.activation(
                    expt[:], pmt_b[:], AF.Exp, accum_out=den[:, b : b + 1]
                )
                # numerator: exp(MT[j,j]) via identity mask + free-dim reduce
                numt = scratch.tile([128, 128], f32, tag="numt")
                nc.gpsimd.tensor_mul(numt[:], expt[:, 0:128], ident_f32)
                nc.vector.reduce_sum(
                    out=num[:, b : b + 1], in_=numt[:], axis=AX.X
                )

            # ---- loss tail: p = num/den; sum log(p + 1e-3) ----
            rden = singles.tile([128, BC], f32)
            nc.vector.reciprocal(rden[:], den[:])
            prob = singles.tile([128, BC], f32)
            nc.vector.tensor_mul(prob[:], num[:], rden[:])
            logp = singles.tile([128, BC], f32)
            nc.scalar.activation(logp[:], prob[:], AF.Ln, bias=c3_sb[:])
            colred = singles.tile([128, 1], f32)
            nc.vector.reduce_sum(out=colred[:], in_=logp[:], axis=AX.X)
            ptot = pcol.tile([1, 1], f32, tag="pc")
            nc.tensor.matmul(
                out=ptot[:], lhsT=colred[:], rhs=ones[:], start=True, stop=True
            )
            out_sb = singles.tile([1, 1], f32)
            nc.vector.tensor_copy(out_sb[:], ptot[:])
            nc.sync.dma_start(out_d.ap(), out_sb[:])

    nc.compile()
    return nc


def _prep_in_maps(z, c, mask, neg_map, W, ln_weight):
    z = np.asarray(z, dtype=np.float32)
    c = np.asarray(c, dtype=np.float32)
    mask = np.asarray(mask).astype(np.int64)
    neg_map = np.asarray(neg_map).astype(np.int64)
    W = np.asarray(W, dtype=np.float32)
    ln_weight = np.asarray(ln_weight, dtype=np.float32)

    wt = np.ascontiguousarray((ln_weight[:, None] * W).T)  # [c, z] = W'[z, c]
    boff = np.arange(BC, dtype=np.int64)[None, :]
    in_maps = []
    for i in range(NCORES):
        bsl = slice(i * BC, (i + 1) * BC)
        zf = np.ascontiguousarray(z[:, bsl, :]).reshape(SEQ * BC, ZD)
        cf = np.ascontiguousarray(c[:, bsl, :]).reshape(SEQ * BC, CD)
        m = mask[:, bsl]
        n = neg_map[:, bsl]
        pos_idx = (m * BC + boff).astype(np.int32)
        neg_idx = (n * BC + boff).astype(np.int32)
        hit = (n[:, None, :] == m[None, :, :]).any(axis=1)  # [L, BC]
        keep = (~hit).astype(np.float32)
        meta = np.concatenate(
            [pos_idx, neg_idx, keep.view(np.int32)], axis=1
        ).astype(np.int32)
        in_maps.append({"zf": zf, "cf": cf, "meta": meta, "wt": wt})
    return in_maps


def kernel(z, c, mask, neg_map, W, ln_weight, ln_bias):
    from concourse import bass_utils

    global _cached
    if _cached is None:
        _cached = _build_program()
    nc = _cached

    in_maps = _prep_in_maps(z, c, mask, neg_map, W, ln_weight)
    res = bass_utils.run_bass_kernel_spmd(
        nc, in_maps, core_ids=list(range(NCORES))
    )
    total = np.float64(0.0)
    for i in range(NCORES):
        total += np.float64(res.results[i]["out"][0, 0])
    return np.float32(-(total / (L * B)))


# revision 16
# speedup vs baseline: 1.0560x; 1.0464x over previous
"""CPC loss kernel for Trainium2, batch-sharded across 8 NeuronCores.

Shapes (hardcoded per problem spec):
  z, c: [2048, 64, 128] f32;  mask, neg_map: [128, 64] int;  W: [128, 128] f32
  ln_weight/ln_bias: [128] f32.  Output: scalar f32.

Per-core plan (Bc = 8 batch elements):
  - Host packs per-core flat row tables zf/cf [SEQ*Bc, 128] plus an int32
    meta tensor holding gather indices and the neg-row keep multiplier.
  - Device gathers only the 3*L*Bc needed rows via indirect DMA (~1.5MB read
    instead of 16MB streamed per core).
  - LN stats per 128-wide segment via bn_stats/bn_aggr; normalization fused
    into one tensor_scalar per segment, split across DVE and GpSimd.
  - PE transposes + two fp32r matmul stages give MT[j, i] = M[i, j, b].
  - exp on ACT with accum_out produces the softmax denominator; the diagonal
    numerator is exp(MT)[:, :128] * I reduced along the free dim.
  - Per-core output is sum_{j,b} log(p_diag + 1e-3); host combines.

ln_weight is folded into W on the host; ln_bias adds a per-(j,b) constant to
every logit column and cancels in the softmax, so it is dropped.  The keep
multiplier (zeroing negatives that collide with mask positions) folds into
rstd.  Softmax needs no max-subtraction: |logits| < ~70 and exp stays well
inside f32 range.
"""

import numpy as np

SEQ, B, L, ZD, CD = 2048, 64, 128, 128, 128
NCORES = 8
BC = B // NCORES  # 8
NSEG = 2 * BC  # 16 LN segments per core (pos + neg)
LN_EPS = 1e-5
TR_F32R = True  # fp32r PE transposes (1.5 cyc/row vs 2.0 for fp32)

_cached = None


def _build_program():
    import concourse.bacc as bacc
    import concourse.tile as tile
    from concourse import bass, mybir
    from concourse.masks import make_identity

    f32 = mybir.dt.float32
    f32r = mybir.dt.float32r
    i32 = mybir.dt.int32
    AF = mybir.ActivationFunctionType
    ALU = mybir.AluOpType
    AX = mybir.AxisListType
    trdt = f32r if TR_F32R else f32

    nc = bacc.Bacc(
        "TRN2",
        target_bir_lowering=False,
        debug=False,
        enable_asserts=True,
        num_devices=NCORES,
    )

    zf_d = nc.dram_tensor("zf", [SEQ * BC, ZD], f32, kind="ExternalInput")
    cf_d = nc.dram_tensor("cf", [SEQ * BC, CD], f32, kind="ExternalInput")
    # meta: [:,0:8]=pos idx, [:,8:16]=neg idx, [:,16:24]=keep (f32 bits)
    meta_d = nc.dram_tensor("meta", [L, 3 * BC], i32, kind="ExternalInput")
    wt_d = nc.dram_tensor("wt", [CD, ZD], f32r, kind="ExternalInput")
    out_d = nc.dram_tensor("out", [1, 1], f32, kind="ExternalOutput")

    with tile.TileContext(nc) as tc:
        with (
            tc.tile_pool(name="singles", bufs=1) as singles,
            tc.tile_pool(name="scratch", bufs=2) as scratch,
            tc.tile_pool(name="pwide", bufs=2, space="PSUM") as pwide,
            tc.tile_pool(name="pzt", bufs=2, space="PSUM") as ppzt,
            tc.tile_pool(name="pmt", bufs=2, space="PSUM") as ppmt,
            tc.tile_pool(name="pcol", bufs=1, space="PSUM") as pcol,
        ):
            # ---- small inputs first: gathers depend on meta ----
            meta_sb = singles.tile([L, 3 * BC], i32)
            nc.sync.dma_start(meta_sb[:], meta_d.ap())
            wt_sb = singles.tile([CD, ZD], f32r)
            nc.sync.dma_start(wt_sb[:], wt_d.ap())

            pos_idx = meta_sb[:, 0:BC]
            neg_idx = meta_sb[:, BC : 2 * BC]
            keep = meta_sb[:, 2 * BC : 3 * BC].bitcast(f32)

            # ---- gathers (c first: it unblocks the PE ct-transpose path) ----
            zall = singles.tile([128, NSEG * ZD], f32)  # [j | seg*128+z]
            call = singles.tile([128, BC * CD], trdt)
            nc.gpsimd.indirect_dma_start(
                out=call[:],
                out_offset=None,
                in_=cf_d.ap(),
                in_offset=bass.IndirectOffsetOnAxis(ap=pos_idx, axis=0),
            )
            nc.gpsimd.indirect_dma_start(
                out=zall[:, 0 : BC * ZD],
                out_offset=None,
                in_=zf_d.ap(),
                in_offset=bass.IndirectOffsetOnAxis(ap=pos_idx, axis=0),
            )
            nc.gpsimd.indirect_dma_start(
                out=zall[:, BC * ZD : NSEG * ZD],
                out_offset=None,
                in_=zf_d.ap(),
                in_offset=bass.IndirectOffsetOnAxis(ap=neg_idx, axis=0),
            )

            # ---- constants ----
            ident = singles.tile([128, 128], f32)
            make_identity(nc, ident[:])
            ident_f32 = ident[:]
            if TR_F32R:
                ident_r = singles.tile([128, 128], f32r)
                nc.gpsimd.tensor_copy(ident_r[:], ident[:])
                ident_tr = ident_r[:]
            else:
                ident_tr = ident[:]
            ones = singles.tile([128, 1], f32)
            nc.vector.memset(ones[:], 1.0)
            eps_sb = singles.tile([128, 1], f32)
            nc.vector.memset(eps_sb[:], LN_EPS)
            c3_sb = singles.tile([128, 1], f32)
            nc.vector.memset(c3_sb[:], 1e-3)

            # ---- c-side: transpose ct per batch, batched E = W' @ ct^T ----
            ctT_sb = singles.tile([CD, BC * L], f32r)
            for g in range(2):
                pct = pwide.tile([128, 512], trdt, tag="pw")
                for k in range(4):
                    b = g * 4 + k
                    nc.tensor.transpose(
                        out=pct[:, k * 128 : (k + 1) * 128],
                        in_=call[:, b * CD : (b + 1) * CD],
                        identity=ident_tr,
                    )
                nc.scalar.copy(
                    ctT_sb[:, g * 512 : (g + 1) * 512], pct[:].bitcast(f32r)
                )
            e_sb = singles.tile([ZD, BC * L], f32r)
            for g in range(2):
                pe = pwide.tile([128, 512], f32, tag="pw")
                nc.tensor.matmul(
                    out=pe[:],
                    lhsT=wt_sb[:],
                    rhs=ctT_sb[:, g * 512 : (g + 1) * 512],
                    start=True,
                    stop=True,
                )
                nc.scalar.copy(e_sb[:, g * 512 : (g + 1) * 512], pe[:])

            # ---- layernorm stats: wide free-dim reduces per 128-segment ----
            z3 = zall[:].rearrange("p (s d) -> p s d", d=ZD)
            s1 = singles.tile([128, NSEG], f32)
            nc.vector.reduce_sum(out=s1[:], in_=z3, axis=AX.X)
            sq = singles.tile([128, NSEG * ZD], f32)
            nc.vector.tensor_mul(sq[:], zall[:], zall[:])
            s2 = singles.tile([128, NSEG], f32)
            nc.vector.reduce_sum(
                out=s2[:], in_=sq[:].rearrange("p (s d) -> p s d", d=ZD), axis=AX.X
            )
            mu = singles.tile([128, NSEG], f32)
            nc.vector.tensor_scalar_mul(mu[:], s1[:], 1.0 / ZD)
            musq = singles.tile([128, NSEG], f32)
            nc.vector.tensor_mul(musq[:], mu[:], mu[:])
            var = singles.tile([128, NSEG], f32)
            nc.vector.tensor_scalar_mul(var[:], s2[:], 1.0 / ZD)
            nc.vector.tensor_tensor(
                out=var[:], in0=var[:], in1=musq[:], op=ALU.subtract
            )
            # rstd = exp(-0.5 * ln(var + eps)): keeps ACT tables to {Copy,Ln,Exp}
            lv = singles.tile([128, NSEG], f32)
            nc.scalar.activation(lv[:], var[:], AF.Ln, bias=eps_sb[:])
            rstd = singles.tile([128, NSEG], f32)
            nc.scalar.activation(rstd[:], lv[:], AF.Exp, scale=-0.5)
            # zero out masked negative rows by folding keep into rstd
            nc.vector.tensor_mul(rstd[:, BC:NSEG], rstd[:, BC:NSEG], keep)

            # ---- normalize: (z - mu) * rstd via two wide broadcast ops ----
            zln = singles.tile([128, NSEG * ZD], trdt)
            zl3 = zln[:].rearrange("p (s d) -> p s d", d=ZD)
            mu_bc = mu[:].unsqueeze(-1).to_broadcast([128, NSEG, ZD])
            rstd_bc = rstd[:].unsqueeze(-1).to_broadcast([128, NSEG, ZD])
            nc.vector.tensor_tensor(out=zl3, in0=z3, in1=mu_bc, op=ALU.subtract)
            nc.vector.tensor_tensor(out=zl3, in0=zl3, in1=rstd_bc, op=ALU.mult)

            # ---- per-batch: transposes, MT matmul, exp/den; diag via PE ----
            den = singles.tile([128, BC], f32)
            pdg = pcol.tile([128, BC], f32, tag="pdg")
            for b in range(BC):
                pzt = ppzt.tile([128, 256], trdt, tag="pzt")
                for i, s in enumerate((b, BC + b)):
                    nc.tensor.transpose(
                        out=pzt[:, i * 128 : (i + 1) * 128],
                        in_=zln[:, s * ZD : (s + 1) * ZD],
                        identity=ident_tr,
                    )
                zt_sb = scratch.tile([128, 256], f32r, tag="zt")
                nc.scalar.copy(zt_sb[:], pzt[:].bitcast(f32r))

                pmt_b = ppmt.tile([128, 256], f32, tag="pmt")
                nc.tensor.matmul(
                    out=pmt_b[:],
                    lhsT=e_sb[:, b * L : (b + 1) * L],
                    rhs=zt_sb[:],
                    start=True,
                    stop=True,
                )
                expt = scratch.tile([128, 256], f32, tag="expt")
                nc.scalar.activation(
                    expt[:], pmt_b[:], AF.Exp, accum_out=den[:, b : b + 1]
                )
                # raw diagonal logit M[j,j] = sum_z E[z,j] * zT_pos[z,j]
                dz = scratch.tile([128, 128], f32, tag="dz")
                nc.vector.tensor_mul(
                    dz[:],
                    e_sb[:, b * L : (b + 1) * L].bitcast(f32),
                    zt_sb[:, 0:128].bitcast(f32),
                )
                nc.tensor.matmul(
                    out=pdg[:, b : b + 1],
                    lhsT=dz[:],
                    rhs=ones[:],
                    start=True,
                    stop=True,
                )
            diagm = singles.tile([128, BC], f32)
            nc.vector.tensor_copy(diagm[:], pdg[:])
            num = singles.tile([128, BC], f32)
            nc.scalar.activation(num[:], diagm[:], AF.Exp)

            # ---- loss tail: p = num/den; sum log(p + 1e-3) ----
            rden = singles.tile([128, BC], f32)
            nc.vector.reciprocal(rden[:], den[:])
            prob = singles.tile([128, BC], f32)
            nc.vector.tensor_mul(prob[:], num[:], rden[:])
            logp = singles.tile([128, BC], f32)
            nc.scalar.activation(logp[:], prob[:], AF.Ln, bias=c3_sb[:])
            colred = singles.tile([128, 1], f32)
            nc.vector.reduce_sum(out=colred[:], in_=logp[:], axis=AX.X)
            ptot = pcol.tile([1, 1], f32, tag="pc")
            nc.tensor.matmul(
                out=ptot[:], lhsT=colred[:], rhs=ones[:], start=True, stop=True
            )
            out_sb = singles.tile([1, 1], f32)
            nc.vector.tensor_copy(out_sb[:], ptot[:])
            nc.sync.dma_start(out_d.ap(), out_sb[:])

    nc.compile()
    return nc


def _prep_in_maps(z, c, mask, neg_map, W, ln_weight):
    z = np.asarray(z, dtype=np.float32)
    c = np.asarray(c, dtype=np.float32)
    mask = np.asarray(mask).astype(np.int64)
    neg_map = np.asarray(neg_map).astype(np.int64)
    W = np.asarray(W, dtype=np.float32)
    ln_weight = np.asarray(ln_weight, dtype=np.float32)

    wt = np.ascontiguousarray((ln_weight[:, None] * W).T)  # [c, z] = W'[z, c]
    boff = np.arange(BC, dtype=np.int64)[None, :]
    in_maps = []
    for i in range(NCORES):
        bsl = slice(i * BC, (i + 1) * BC)
        zf = np.ascontiguousarray(z[:, bsl, :]).reshape(SEQ * BC, ZD)
        cf = np.ascontiguousarray(c[:, bsl, :]).reshape(SEQ * BC, CD)
        m = mask[:, bsl]
        n = neg_map[:, bsl]
        pos_idx = (m * BC + boff).astype(np.int32)
        neg_idx = (n * BC + boff).astype(np.int32)
        hit = (n[:, None, :] == m[None, :, :]).any(axis=1)  # [L, BC]
        keep = (~hit).astype(np.float32)
        meta = np.concatenate(
            [pos_idx, neg_idx, keep.view(np.int32)], axis=1
        ).astype(np.int32)
        in_maps.append({"zf": zf, "cf": cf, "meta": meta, "wt": wt})
    return in_maps


def kernel(z, c, mask, neg_map, W, ln_weight, ln_bias):
    from concourse import bass_utils

    global _cached
    if _cached is None:
        _cached = _build_program()
    nc = _cached

    in_maps = _prep_in_maps(z, c, mask, neg_map, W, ln_weight)
    res = bass_utils.run_bass_kernel_spmd(
        nc, in_maps, core_ids=list(range(NCORES))
    )
    total = np.float64(0.0)
    for i in range(NCORES):
        total += np.float64(res.results[i]["out"][0, 0])
    return np.float32(-(total / (L * B)))


# revision 18
# speedup vs baseline: 1.1933x; 1.1300x over previous
"""CPC loss kernel for Trainium2, batch-sharded across 8 NeuronCores.

Shapes (hardcoded per problem spec):
  z, c: [2048, 64, 128] f32;  mask, neg_map: [128, 64] int;  W: [128, 128] f32
  ln_weight/ln_bias: [128] f32.  Output: scalar f32.

Per-core plan (Bc = 8 batch elements):
  - Host packs per-core flat row tables zf/cf [SEQ*Bc, 128], an int32 meta
    tensor (gather indices + keep multiplier), and wpack = [W'^T | I].
  - Device gathers only the 3*L*Bc needed rows via indirect DMA (~1.5MB read
    instead of 16MB streamed per core); gpsimd does nothing but DMA so the
    gathers issue as early as possible.
  - LN stats via wide free-dim reduces; rstd = Newton rsqrt on DVE (no ACT
    table); normalize via per-segment ACT Identity with scale/bias vectors.
  - PE transposes + two fp32r matmul stages give MT[j, i] = M[i, j, b].
  - exp on ACT (the only transcendental table) with accum_out producing the
    softmax denominator; numerator = diag of exp(MT) via identity mask.
  - Device outputs num/den [128, 2*Bc]; host does log(num/den + 1e-3) and
    the mean in float64.

ln_weight is folded into W on the host; ln_bias adds a per-(j,b) constant to
every logit column and cancels in the softmax, so it is dropped.  The keep
multiplier (zeroing negatives that collide with mask positions) folds into
rstd.  Softmax needs no max-subtraction: |logits| < ~70 stays in f32 range.
"""

import numpy as np

SEQ, B, L, ZD, CD = 2048, 64, 128, 128, 128
NCORES = 8
BC = B // NCORES  # 8
NSEG = 2 * BC  # 16 LN segments per core (pos + neg)
LN_EPS = 1e-5
RSQRT_MAGIC = 0x5F3759DF

_cached = None


def _build_program():
    import concourse.bacc as bacc
    import concourse.tile as tile
    from concourse import bass, mybir

    f32 = mybir.dt.float32
    f32r = mybir.dt.float32r
    i32 = mybir.dt.int32
    AF = mybir.ActivationFunctionType
    ALU = mybir.AluOpType
    AX = mybir.AxisListType

    nc = bacc.Bacc(
        "TRN2",
        target_bir_lowering=False,
        debug=False,
        enable_asserts=True,
        num_devices=NCORES,
    )

    zf_d = nc.dram_tensor("zf", [SEQ * BC, ZD], f32, kind="ExternalInput")
    cf_d = nc.dram_tensor("cf", [SEQ * BC, CD], f32, kind="ExternalInput")
    # meta: [:,0:8]=pos idx, [:,8:16]=neg idx, [:,16:24]=keep (f32 bits)
    meta_d = nc.dram_tensor("meta", [L, 3 * BC], i32, kind="ExternalInput")
    # wpack: [:,0:128] = W'^T (f32r), [:,128:256] = identity
    wpack_d = nc.dram_tensor("wpack", [128, 256], f32r, kind="ExternalInput")
    out_d = nc.dram_tensor("out", [128, NSEG], f32, kind="ExternalOutput")

    with tile.TileContext(nc) as tc:
        with (
            tc.tile_pool(name="singles", bufs=1) as singles,
            tc.tile_pool(name="scratch", bufs=3) as scratch,
            tc.tile_pool(name="pwide", bufs=2, space="PSUM") as pwide,
            tc.tile_pool(name="pzt", bufs=3, space="PSUM") as ppzt,
            tc.tile_pool(name="pmt", bufs=3, space="PSUM") as ppmt,
        ):
            # ---- small inputs first: gathers depend only on meta ----
            meta_sb = singles.tile([L, 3 * BC], i32)
            nc.sync.dma_start(meta_sb[:], meta_d.ap())
            wpack_sb = singles.tile([128, 256], f32r)
            nc.sync.dma_start(wpack_sb[:], wpack_d.ap())
            wt_sb = wpack_sb[:, 0:128]
            ident_r = wpack_sb[:, 128:256]
            ident_f32 = ident_r.bitcast(f32)

            pos_idx = meta_sb[:, 0:BC]
            neg_idx = meta_sb[:, BC : 2 * BC]
            keep = meta_sb[:, 2 * BC : 3 * BC].bitcast(f32)

            # ---- gathers (c first: it unblocks the PE ct-transpose path) ----
            zall = singles.tile([128, NSEG * ZD], f32)  # [j | seg*128+z]
            call = singles.tile([128, BC * CD], f32r)
            nc.gpsimd.indirect_dma_start(
                out=call[:],
                out_offset=None,
                in_=cf_d.ap(),
                in_offset=bass.IndirectOffsetOnAxis(ap=pos_idx, axis=0),
            )
            nc.gpsimd.indirect_dma_start(
                out=zall[:, 0 : BC * ZD],
                out_offset=None,
                in_=zf_d.ap(),
                in_offset=bass.IndirectOffsetOnAxis(ap=pos_idx, axis=0),
            )
            nc.gpsimd.indirect_dma_start(
                out=zall[:, BC * ZD : NSEG * ZD],
                out_offset=None,
                in_=zf_d.ap(),
                in_offset=bass.IndirectOffsetOnAxis(ap=neg_idx, axis=0),
            )

            # ---- c-side: transpose ct per batch, batched E = W' @ ct^T ----
            ctT_sb = singles.tile([CD, BC * L], f32r)
            for g in range(2):
                pct = pwide.tile([128, 512], f32r, tag="pw")
                for k in range(4):
                    b = g * 4 + k
                    nc.tensor.transpose(
                        out=pct[:, k * 128 : (k + 1) * 128],
                        in_=call[:, b * CD : (b + 1) * CD],
                        identity=ident_r,
                    )
                nc.scalar.copy(ctT_sb[:, g * 512 : (g + 1) * 512], pct[:])
            e_sb = singles.tile([ZD, BC * L], f32r)
            for g in range(2):
                pe = pwide.tile([128, 512], f32, tag="pw")
                nc.tensor.matmul(
                    out=pe[:],
                    lhsT=wt_sb,
                    rhs=ctT_sb[:, g * 512 : (g + 1) * 512],
                    start=True,
                    stop=True,
                )
                nc.scalar.copy(e_sb[:, g * 512 : (g + 1) * 512], pe[:])

            # ---- layernorm stats: wide free-dim reduces per 128-segment ----
            z3 = zall[:].rearrange("p (s d) -> p s d", d=ZD)
            s1 = singles.tile([128, NSEG], f32)
            nc.vector.reduce_sum(out=s1[:], in_=z3, axis=AX.X)
            sq = singles.tile([128, NSEG * ZD], f32)
            nc.scalar.activation(sq[:], zall[:], AF.Square)
            s2 = singles.tile([128, NSEG], f32)
            nc.vector.reduce_sum(
                out=s2[:], in_=sq[:].rearrange("p (s d) -> p s d", d=ZD), axis=AX.X
            )
            mu = singles.tile([128, NSEG], f32)
            nc.vector.tensor_scalar_mul(mu[:], s1[:], 1.0 / ZD)
            musq = singles.tile([128, NSEG], f32)
            nc.vector.tensor_mul(musq[:], mu[:], mu[:])
            var = singles.tile([128, NSEG], f32)
            nc.vector.tensor_scalar_mul(var[:], s2[:], 1.0 / ZD)
            nc.vector.tensor_tensor(
                out=var[:], in0=var[:], in1=musq[:], op=ALU.subtract
            )

            # ---- rstd = rsqrt(var + eps): Newton iteration, DVE only ----
            vv = singles.tile([128, NSEG], f32)
            nc.vector.tensor_scalar(
                out=vv[:], in0=var[:], scalar1=1.0, scalar2=LN_EPS,
                op0=ALU.mult, op1=ALU.add,
            )
            y = singles.tile([128, NSEG], f32)
            yi = y[:].bitcast(i32)
            nc.vector.tensor_scalar(
                out=yi, in0=vv[:].bitcast(i32), scalar1=1, scalar2=None,
                op0=ALU.arith_shift_right,
            )
            nc.vector.tensor_scalar(
                out=yi, in0=yi, scalar1=-1, scalar2=RSQRT_MAGIC,
                op0=ALU.mult, op1=ALU.add,
            )
            t1 = singles.tile([128, NSEG], f32)
            for _ in range(3):
                nc.vector.tensor_mul(t1[:], y[:], y[:])
                nc.vector.tensor_mul(t1[:], t1[:], vv[:])
                nc.vector.tensor_scalar(
                    out=t1[:], in0=t1[:], scalar1=-0.5, scalar2=1.5,
                    op0=ALU.mult, op1=ALU.add,
                )
                nc.vector.tensor_mul(y[:], y[:], t1[:])
            rstd = y

            # zero out masked negative rows by folding keep into rstd
            nc.vector.tensor_mul(rstd[:, BC:NSEG], rstd[:, BC:NSEG], keep)
            # norm bias = -mu * rstd
            nmr = singles.tile([128, NSEG], f32)
            nc.vector.tensor_mul(nmr[:], mu[:], rstd[:])
            nc.vector.tensor_scalar_mul(nmr[:], nmr[:], -1.0)

            # ---- normalize per segment on ACT: z*rstd + (-mu*rstd) ----
            zln = singles.tile([128, NSEG * ZD], f32r)
            zl3 = zln[:].rearrange("p (s d) -> p s d", d=ZD)
            for b in range(BC):
                for s in (b, BC + b):
                    nc.scalar.activation(
                        zl3[:, s, :], z3[:, s, :], AF.Identity,
                        bias=nmr[:, s : s + 1], scale=rstd[:, s : s + 1],
                    )

            # ---- per-batch: transposes, MT matmul, exp -> num/den ----
            outv = singles.tile([128, NSEG], f32)  # [num | den]
            for b in range(BC):
                pzt = ppzt.tile([128, 256], f32r, tag="pzt")
                for i, s in enumerate((b, BC + b)):
                    nc.tensor.transpose(
                        out=pzt[:, i * 128 : (i + 1) * 128],
                        in_=zln[:, s * ZD : (s + 1) * ZD],
                        identity=ident_r,
                    )
                zt_sb = scratch.tile([128, 256], f32r, tag="zt")
                nc.vector.tensor_copy(zt_sb[:], pzt[:])

                pmt_b = ppmt.tile([128, 256], f32, tag="pmt")
                nc.tensor.matmul(
                    out=pmt_b[:],
                    lhsT=e_sb[:, b * L : (b + 1) * L],
                    rhs=zt_sb[:],
                    start=True,
                    stop=True,
                )
                expt = scratch.tile([128, 256], f32, tag="expt")
                nc.scalar.activation(
                    expt[:], pmt_b[:], AF.Exp,
                    accum_out=outv[:, BC + b : BC + b + 1],
                )
                # numerator: diag of exp(MT) via identity mask + reduce
                numt = scratch.tile([128, 128], f32, tag="numt")
                nc.vector.tensor_mul(numt[:], expt[:, 0:128], ident_f32)
                nc.vector.reduce_sum(
                    out=outv[:, b : b + 1], in_=numt[:], axis=AX.X
                )

            nc.sync.dma_start(out_d.ap(), outv[:])

    nc.compile()
    return nc


def _prep_in_maps(z, c, mask, neg_map, W, ln_weight):
    z = np.asarray(z, dtype=np.float32)
    c = np.asarray(c, dtype=np.float32)
    mask = np.asarray(mask).astype(np.int64)
    neg_map = np.asarray(neg_map).astype(np.int64)
    W = np.asarray(W, dtype=np.float32)
    ln_weight = np.asarray(ln_weight, dtype=np.float32)

    wt = (ln_weight[:, None] * W).T  # [c, z] = W'[z, c]
    wpack = np.ascontiguousarray(
        np.concatenate([wt, np.eye(128, dtype=np.float32)], axis=1)
    )
    boff = np.arange(BC, dtype=np.int64)[None, :]
    in_maps = []
    for i in range(NCORES):
        bsl = slice(i * BC, (i + 1) * BC)
        zf = np.ascontiguousarray(z[:, bsl, :]).reshape(SEQ * BC, ZD)
        cf = np.ascontiguousarray(c[:, bsl, :]).reshape(SEQ * BC, CD)
        m = mask[:, bsl]
        n = neg_map[:, bsl]
        pos_idx = (m * BC + boff).astype(np.int32)
        neg_idx = (n * BC + boff).astype(np.int32)
        hit = (n[:, None, :] == m[None, :, :]).any(axis=1)  # [L, BC]
        keep = (~hit).astype(np.float32)
        meta = np.concatenate(
            [pos_idx, neg_idx, keep.view(np.int32)], axis=1
        ).astype(np.int32)
        in_maps.append({"zf": zf, "cf": cf, "meta": meta, "wpack": wpack})
    return in_maps


def _combine(results):
    total = np.float64(0.0)
    for r in results:
        o = np.asarray(r["out"], dtype=np.float64)
        num, den = o[:, 0:BC], o[:, BC : 2 * BC]
        total += np.log(num / den + 1e-3).sum()
    return np.float32(-(total / (L * B)))


def kernel(z, c, mask, neg_map, W, ln_weight, ln_bias):
    from concourse import bass_utils

    global _cached
    if _cached is None:
        _cached = _build_program()
    nc = _cached

    in_maps = _prep_in_maps(z, c, mask, neg_map, W, ln_weight)
    res = bass_utils.run_bass_kernel_spmd(
        nc, in_maps, core_ids=list(range(NCORES))
    )
    return _combine(res.results)
